# revision 20
# baseline (speedup 1.0000x reference)
"""Trainium2 Bass kernel for nn_ClassBlock (dense_transformer, memory regime).

Strategy
--------
The ClassBlock only transforms x[:, 0, :] (the cls token); x[:, 1:, :] passes
through untouched.  The kernel is therefore dominated by a 256 MB HBM->HBM
copy.  Sharding:
  * batch-parallel over 8 cores (2 batches/core) for the pass-through copy,
    split across two DMA streams (SP HWDGE ring + SWDGE) to saturate HBM,
  * the cls compute ([16,1024] activations) is replicated on every core,
    except the heavy MLP weights: fc1 is column-sharded, fc2 row-sharded,
  * MLP partials are exchanged with an AllToAll (pure p2p, much lower latency
    than ReduceScatter) and reduced locally on the PE,
  * each core writes only its own 2 batch rows of the cls result.
All matmuls run in bf16 (4x PE rate, half the weight HBM bytes); the row
select of cls1 stays fp32 (exact).  LayerNorm runs entirely on the vector
engine (bit-trick rsqrt + Newton) so the ACT LUT never thrashes.
L=1 structural simplifications (3x3 'SAME' depthwise conv on a 1x1 map ==
center tap; selective scan with L=1, h0=0 == dBu*Cs + D*u).
"""

import numpy as np

B, NTOK, C = 16, 4097, 1024
NCORES = 8
BPC = B // NCORES            # batches per core
DG = C // 4                  # 256 per-group channels
DTRANK = 16
HID = 4 * C                  # 4096
RED = C // 16                # 64
FC1_SH = HID // NCORES       # 512 fc1 column shard
FC2_SH = HID // NCORES       # 512 fc2 row shard
EPS = 1e-5

# packed per-channel vector blob rows (each row = 1024 f32)
R_GMW, R_GMB, R_SE2B, R_N1W, R_N1B, R_N2W, R_N2B, R_FC2B, R_GMPB = range(9)
R_CW, R_CB, R_DTB, R_D, R_ONW, R_ONB, R_MISC = range(9, 16)
NV = 16
# misc row layout: [64:128]=se_fc1_b, [512:1024]=fc1_b shard
OFF_SE1B = 64
OFF_FC1B = 512

# copy split: SYNC_N one-MB tiles on the SP HWDGE ring; B_N half-MB tiles on
# SWDGE (issued early, sized small so the collective dispatch behind them
# fires with little inter-core skew); C_N half-MB tiles on the ACT HWDGE
# ring after the cls chain's scalar ops retire.
CPF = 2048                   # 8 KB/partition per sync staging tile
CPH = 1024                   # 4 KB/partition per B/C staging tile
NCP = (NTOK - 1) * C // (128 * CPF)  # 16 tiles per batch row
NALL = NCP * BPC             # 32 one-MB tile slots
SYNC_N = 22
B_N = 8                      # half-MB tiles (4 MB) for SWDGE
C_N = 12                     # half-MB tiles (6 MB) for scalar post-chain
LOOKA = 4                    # sync-stream load lookahead
LOOKB = 1                    # swdge-stream load lookahead

DEBUG_TAPS = False

_CACHE = {}
LAST_RESULT = None
TRACE = False


def _f32(a):
    return np.ascontiguousarray(np.asarray(a, dtype=np.float32))


def _build(debug_taps):
    import concourse.bass as bass
    import concourse.tile as tile
    from concourse import bacc, mybir

    f32 = mybir.dt.float32
    i32 = mybir.dt.int32
    wdt = mybir.dt.bfloat16
    AF = mybir.ActivationFunctionType
    ALU = mybir.AluOpType

    # Bacc (not plain Bass): its compile() legalizes to <=1 sync wait per
    # instruction (generate_event_semaphores), which TRN2 codegen requires.
    nc = bacc.Bacc("TRN2", target_bir_lowering=False, num_devices=NCORES)

    # ---- I/O ------------------------------------------------------------
    xs_h = nc.dram_tensor("xs", [BPC, NTOK, C], f32, kind="ExternalInput")
    cls_h = nc.dram_tensor("cls_all", [B, C], f32, kind="ExternalInput")
    sel_h = nc.dram_tensor("sel", [B, 2 * BPC], f32, kind="ExternalInput")
    id_h = nc.dram_tensor("ident16", [B, B], f32, kind="ExternalInput")
    vecs_h = nc.dram_tensor("vecs", [B, NV * 1024], f32, kind="ExternalInput")
    # weights arrive pre-relaid by the host as [128, tiles, N] (partition
    # contiguous) — partition-scattered rearranges at load time make HWDGE
    # emit thousands of tiny descriptors (~28 GB/s measured)
    se1w_h = nc.dram_tensor("se1w", [128, 8, RED], wdt, kind="ExternalInput")
    se2w_h = nc.dram_tensor("se2w", [RED, C], wdt, kind="ExternalInput")
    ipw_h = nc.dram_tensor("ipw", [128, 8, 512], wdt, kind="ExternalInput")
    xpw_h = nc.dram_tensor("xpw", [128, 8, DTRANK + 2], wdt, kind="ExternalInput")
    dtw_h = nc.dram_tensor("dtw", [DTRANK, 4, DG], wdt, kind="ExternalInput")
    opw_h = nc.dram_tensor("opw", [128, 8, DG], wdt, kind="ExternalInput")
    gmw_h = nc.dram_tensor("gmw", [128, 8, C], wdt, kind="ExternalInput")
    fc1_h = nc.dram_tensor("fc1s", [128, 8, FC1_SH], wdt, kind="ExternalInput")
    fc2_h = nc.dram_tensor("fc2s", [128, 4, C], wdt, kind="ExternalInput")
    out_h = nc.dram_tensor("out", [BPC, NTOK, C], f32, kind="ExternalOutput")
    dbg_h = None
    if debug_taps:
        dbg_h = nc.dram_tensor("dbg", [8, B, C], f32, kind="ExternalOutput")

    from contextlib import ExitStack

    with tile.TileContext(nc) as tc, ExitStack() as ctx:
        singles = ctx.enter_context(tc.tile_pool(name="singles", bufs=1))
        wst = ctx.enter_context(tc.tile_pool(name="wst", bufs=8))
        a1k = ctx.enter_context(tc.tile_pool(name="a1k", bufs=2))
        a512 = ctx.enter_context(tc.tile_pool(name="a512", bufs=2))
        tiny = ctx.enter_context(tc.tile_pool(name="tiny", bufs=1))
        tp = ctx.enter_context(tc.tile_pool(name="tp", bufs=1))
        stats = ctx.enter_context(tc.tile_pool(name="stats", bufs=3))
        ppt = ctx.enter_context(tc.tile_pool(name="ppt", bufs=4, space="PSUM"))
        pm = ctx.enter_context(tc.tile_pool(name="pm", bufs=2, space="PSUM"))
        dram = ctx.enter_context(tc.tile_pool(name="dram", bufs=1, space="DRAM"))
        cpa = ctx.enter_context(tc.tile_pool(name="cpa", bufs=4))
        cpb = ctx.enter_context(tc.tile_pool(name="cpb", bufs=2))
        cpc = cpb  # B (gpsimd, 0-100us) and C (scalar, 150us+) never overlap

        # ---- the big pass-through copy (bulk of the kernel) -------------
        # DRAM->DRAM DMA is latency-bound, so stage through SBUF.  Tiles
        # 0..SYNC_N-1 ride the SP HWDGE ring; the rest ride SWDGE (issued
        # after the big-weight loads below, sized to finish issuing before
        # the collective dispatch at the end of the gpsimd stream).
        xs_flat = xs_h[:].rearrange("b t c -> b (t c)")
        out_flat = out_h[:].rearrange("b t c -> b (t c)")
        srcs, dsts, srcs_h, dsts_h = [], [], [], []
        for b in range(BPC):
            srcs.append(xs_flat[b, C:].rearrange("(n p f) -> n p f", p=128, f=CPF))
            dsts.append(out_flat[b, C:].rearrange("(n p f) -> n p f", p=128, f=CPF))
            srcs_h.append(xs_flat[b, C:].rearrange("(n p f) -> n p f", p=128, f=CPH))
            dsts_h.append(out_flat[b, C:].rearrange("(n p f) -> n p f", p=128, f=CPH))

        def copy_stream(eng, pool, idxs, looka, half=False):
            ss, dd, npr, w = ((srcs_h, dsts_h, 2 * NCP, CPH) if half
                             else (srcs, dsts, NCP, CPF))
            tiles = {}
            n_ = len(idxs)
            for k in range(n_ + looka):
                if k < n_:
                    t = pool.tile([128, w], f32, tag="cp")
                    i = idxs[k]
                    eng.dma_start(out=t[:], in_=ss[i // npr][i % npr])
                    tiles[k] = t
                if k >= looka:
                    i = idxs[k - looka]
                    eng.dma_start(out=dd[i // npr][i % npr],
                                  in_=tiles.pop(k - looka)[:])

        # half-MB tile index space: MB-tile slots SYNC_N..NALL map to half
        # tiles 2*SYNC_N..2*NALL; B gets the first B_N, C the rest.
        copy_stream(nc.sync, cpa, list(range(SYNC_N)), LOOKA)

        # ---- small weights / activations: scalar (ACT HWDGE) queue ------
        # Issued first so the cls chain can start within a few us.
        vecs = singles.tile([B, NV * 1024], f32, tag="vecs")
        nc.scalar.dma_start(out=vecs[:], in_=vecs_h[:])
        cls_t = singles.tile([B, C], f32, tag="cls")
        nc.scalar.dma_start(out=cls_t[:], in_=cls_h[:])
        sel_t = singles.tile([B, 2 * BPC], f32, tag="sel")
        nc.scalar.dma_start(out=sel_t[:], in_=sel_h[:])
        ident = singles.tile([B, B], f32, tag="ident")
        nc.scalar.dma_start(out=ident[:], in_=id_h[:])
        se1w = singles.tile([128, 8, RED], wdt, tag="se1w")
        nc.scalar.dma_start(out=se1w[:], in_=se1w_h[:])
        se2w = singles.tile([RED, 2, 512], wdt, tag="se2w")
        nc.scalar.dma_start(out=se2w[:], in_=se2w_h[:].rearrange("k (c n) -> k c n", c=2))
        xpw = singles.tile([128, 8, DTRANK + 2], wdt, tag="xpw")
        nc.scalar.dma_start(out=xpw[:], in_=xpw_h[:])
        dtw = singles.tile([DTRANK, 4, DG], wdt, tag="dtw")
        nc.scalar.dma_start(out=dtw[:], in_=dtw_h[:])
        ipw = singles.tile([128, 8, 512], wdt, tag="ipw")
        nc.scalar.dma_start(out=ipw[:], in_=ipw_h[:])

        # ---- big weights: SWDGE, then copy-B, then the collective -------
        opw = singles.tile([128, 8, DG], wdt, tag="opw")
        nc.gpsimd.dma_start(out=opw[:], in_=opw_h[:])
        gmw_r = gmw_h[:]     # [128, 8, 1024]
        fc1_r = fc1_h[:]     # [128, 8, 512]
        fc2_r = fc2_h[:]     # [128, 4, 1024]
        gmw_c = []
        for cnum in range(4):
            t = wst.tile([128, 2, 1024], wdt, tag="wst")
            nc.gpsimd.dma_start(out=t[:], in_=gmw_r[:, 2 * cnum:2 * cnum + 2, :])
            gmw_c.append(t)
        fc1_c = []
        for cnum in range(2):
            t = wst.tile([128, 4, 512], wdt, tag="wst")
            nc.gpsimd.dma_start(out=t[:], in_=fc1_r[:, 4 * cnum:4 * cnum + 4, :])
            fc1_c.append(t)
        fc2_c = []
        for cnum in range(2):
            t = wst.tile([128, 2, 1024], wdt, tag="wst")
            nc.gpsimd.dma_start(out=t[:], in_=fc2_r[:, 2 * cnum:2 * cnum + 2, :])
            fc2_c.append(t)

        # copy-B: all weight chunks above issue instantly (wst bufs=8, no
        # recycling), so nothing here is gated on chain progress; keeping
        # copy-B small bounds the inter-core skew of the collective dispatch
        # that follows it in this stream.
        copy_stream(nc.gpsimd, cpb,
                    list(range(2 * SYNC_N, 2 * SYNC_N + B_N)), LOOKB, half=True)

        def vrow(row, n=1024, off=0):
            return vecs[:, row * 1024 + off: row * 1024 + off + n]

        # ---- helpers -----------------------------------------------------
        QK = 0x5F3759DF

        def ln(x_sl, w_sl, b_sl, out_sl, cdim):
            # fully on the vector engine: bn stats + bit-trick rsqrt with 3
            # Newton steps + fused (x-mean)*rstd; no ACT LUT traffic.
            nsub = max(1, cdim // 512)
            if nsub == 1:
                st = stats.tile([B, 6], f32, tag="st6")
                nc.vector.bn_stats(out=st[:], in_=x_sl)
            else:
                st = stats.tile([B, nsub, 6], f32, tag="st26")
                for s in range(nsub):
                    nc.vector.bn_stats(out=st[:, s, :], in_=x_sl[:, s * 512:(s + 1) * 512])
            mv = stats.tile([B, 2], f32, tag="mv")
            nc.vector.bn_aggr(out=mv[:], in_=st[:])
            q = stats.tile([B, 2], f32, tag="qrs")     # col0 = var+eps, col1 = y
            nc.vector.tensor_scalar_add(out=q[:, 0:1], in0=mv[:, 1:2], scalar1=EPS)
            qi = q[:].bitcast(i32)
            nc.vector.tensor_scalar(out=qi[:, 1:2], in0=qi[:, 0:1], scalar1=1,
                                    scalar2=None, op0=ALU.arith_shift_right)
            nc.vector.tensor_scalar(out=qi[:, 1:2], in0=qi[:, 1:2], scalar1=QK,
                                    scalar2=-1, op0=ALU.subtract, op1=ALU.mult)
            a = stats.tile([B, 1], f32, tag="nwt")
            for _ in range(2):
                nc.vector.tensor_mul(out=a[:], in0=q[:, 1:2], in1=q[:, 1:2])
                nc.vector.tensor_mul(out=a[:], in0=a[:], in1=q[:, 0:1])
                nc.vector.tensor_scalar(out=a[:], in0=a[:], scalar1=-0.5,
                                        scalar2=1.5, op0=ALU.mult, op1=ALU.add)
                nc.vector.tensor_mul(out=q[:, 1:2], in0=q[:, 1:2], in1=a[:])
            nc.vector.tensor_scalar(out=out_sl, in0=x_sl, scalar1=mv[:, 0:1],
                                    scalar2=q[:, 1:2], op0=ALU.subtract, op1=ALU.mult)
            nc.vector.tensor_mul(out=out_sl, in0=out_sl, in1=w_sl)
            nc.vector.tensor_add(out=out_sl, in0=out_sl, in1=b_sl)

        def transpose_in(x_sl, cdim, tag="tp"):
            # [16, cdim] (sbuf) -> [128, cdim//128, 16] (sbuf, bf16)
            kt = cdim // 128
            xT = tp.tile([128, kt, B], wdt, tag=tag)
            for t in range(kt):
                pt = ppt.tile([128, B], f32, tag="pt")
                nc.tensor.transpose(pt[:], x_sl[:, t * 128:(t + 1) * 128], ident[:])
                nc.vector.tensor_copy(out=xT[:, t, :], in_=pt[:])
            return xT

        def tap(i, src_sl, n=C):
            if dbg_h is not None:
                nc.scalar.dma_start(out=dbg_h[i, :, :n], in_=src_sl)

        # ---- cls chain ---------------------------------------------------
        xn = singles.tile([B, C], f32, tag="xn")
        ln(cls_t[:], vrow(R_GMW), vrow(R_GMB), xn[:], C)
        tap(0, xn[:])
        xnT = transpose_in(xn[:], C, tag="xnT_p")

        # SE block
        seh_p = pm.tile([B, RED], f32, tag="pm")
        for t in range(8):
            nc.tensor.matmul(seh_p[:], lhsT=xnT[:, t, :], rhs=se1w[:, t, :],
                             start=(t == 0), stop=(t == 7))
        seh = tiny.tile([B, RED], f32, tag="seh")
        nc.vector.tensor_add(out=seh[:], in0=seh_p[:], in1=vrow(R_MISC, RED, OFF_SE1B))
        nc.scalar.activation(out=seh[:], in_=seh[:], func=AF.Relu)
        pt = ppt.tile([128, B], f32, tag="pt")
        nc.tensor.transpose(pt[:RED, :], seh[:], ident[:])
        sehT = tiny.tile([RED, B], wdt, tag="sehT")
        nc.vector.tensor_copy(out=sehT[:], in_=pt[:RED, :])
        se_p = pm.tile([B, C], f32, tag="pm")
        for n in range(2):
            nc.tensor.matmul(se_p[:, n * 512:(n + 1) * 512], lhsT=sehT[:],
                             rhs=se2w[:, n, :], start=True, stop=True)
        se_t = singles.tile([B, C], f32, tag="se")
        nc.vector.tensor_add(out=se_t[:], in0=se_p[:], in1=vrow(R_SE2B))
        nc.scalar.activation(out=se_t[:], in_=se_t[:], func=AF.Sigmoid)
        tap(1, se_t[:])

        # SS2D groups — sigmoid phase: in_proj, u = silu(xs*cw+cb), sz = silu(z)
        ycat = singles.tile([B, C], f32, tag="ycat")
        u_all = singles.tile([B, C], f32, tag="uall")
        sz_all = singles.tile([B, C], f32, tag="szall")
        for g in range(4):
            xz_p = pm.tile([B, 2 * DG], f32, tag="pm")
            for t in range(2):
                gt = 2 * g + t
                nc.tensor.matmul(xz_p[:], lhsT=xnT[:, gt, :], rhs=ipw[:, gt, :],
                                 start=(t == 0), stop=(t == 1))
            sl = slice(g * DG, (g + 1) * DG)
            nc.vector.tensor_copy(out=u_all[:, sl], in_=xz_p[:, :DG])
            nc.vector.tensor_copy(out=sz_all[:, sl], in_=xz_p[:, DG:])
        nc.vector.tensor_mul(out=u_all[:], in0=u_all[:], in1=vrow(R_CW))
        nc.vector.tensor_add(out=u_all[:], in0=u_all[:], in1=vrow(R_CB))
        sgt = a1k.tile([B, C], f32, tag="a1k")
        nc.scalar.activation(out=sgt[:], in_=u_all[:], func=AF.Sigmoid)
        nc.vector.tensor_mul(out=u_all[:], in0=u_all[:], in1=sgt[:])
        sgt2 = a1k.tile([B, C], f32, tag="a1k")
        nc.scalar.activation(out=sgt2[:], in_=sz_all[:], func=AF.Sigmoid)
        nc.vector.tensor_mul(out=sz_all[:], in0=sz_all[:], in1=sgt2[:])

        # x_dbl, delta = softplus, y, out-norm LN
        uT = transpose_in(u_all[:], C, tag="uT8")
        delta_all = singles.tile([B, C], f32, tag="dall")
        bcs = []
        for g in range(4):
            xdb_p = pm.tile([B, DTRANK + 2], f32, tag="pm")
            for t in range(2):
                nc.tensor.matmul(xdb_p[:], lhsT=uT[:, 2 * g + t, :],
                                 rhs=xpw[:, 2 * g + t, :],
                                 start=(t == 0), stop=(t == 1))
            xdb = tiny.tile([B, DTRANK + 2], f32, tag="xdb")
            nc.vector.tensor_copy(out=xdb[:], in_=xdb_p[:])
            bc = stats.tile([B, 1], f32, tag="bc")
            nc.vector.tensor_mul(out=bc[:], in0=xdb[:, DTRANK:DTRANK + 1],
                                 in1=xdb[:, DTRANK + 1:DTRANK + 2])
            bcs.append(bc)
            pt2 = ppt.tile([128, B], f32, tag="pt")
            nc.tensor.transpose(pt2[:DTRANK, :], xdb[:, :DTRANK], ident[:])
            dtsT = tiny.tile([DTRANK, B], wdt, tag="dtsT")
            nc.vector.tensor_copy(out=dtsT[:], in_=pt2[:DTRANK, :])
            dl_p = pm.tile([B, DG], f32, tag="pm")
            nc.tensor.matmul(dl_p[:], lhsT=dtsT[:], rhs=dtw[:, g, :], start=True, stop=True)
            nc.vector.tensor_copy(out=delta_all[:, g * DG:(g + 1) * DG], in_=dl_p[:])
        nc.vector.tensor_add(out=delta_all[:], in0=delta_all[:], in1=vrow(R_DTB))
        # softplus(x) = relu(x) + ln(1 + exp(-|x|)); native Softplus LUT is
        # broken in this neuronx-cc build
        spt = a1k.tile([B, C], f32, tag="a1k")
        nc.scalar.activation(out=spt[:], in_=delta_all[:], func=AF.Abs)
        nc.scalar.activation(out=spt[:], in_=spt[:], func=AF.Exp, scale=-1.0)
        nc.vector.tensor_scalar_add(out=spt[:], in0=spt[:], scalar1=1.0)
        nc.scalar.activation(out=spt[:], in_=spt[:], func=AF.Ln)
        nc.scalar.activation(out=delta_all[:], in_=delta_all[:], func=AF.Relu)
        nc.vector.tensor_add(out=delta_all[:], in0=delta_all[:], in1=spt[:])
        # y = delta*u*(Bs*Cs) + D*u
        nc.vector.tensor_mul(out=delta_all[:], in0=delta_all[:], in1=u_all[:])
        for g in range(4):
            sl2 = slice(g * DG, (g + 1) * DG)
            nc.vector.tensor_scalar_mul(out=delta_all[:, sl2], in0=delta_all[:, sl2],
                                        scalar1=bcs[g][:])
        t2 = a1k.tile([B, C], f32, tag="a1k")
        nc.vector.tensor_mul(out=t2[:], in0=u_all[:], in1=vrow(R_D))
        nc.vector.tensor_add(out=delta_all[:], in0=delta_all[:], in1=t2[:])
        # per-group out-norm LN (stats over 256 channels), then * silu(z)
        for g in range(4):
            sl3 = slice(g * DG, (g + 1) * DG)
            ln(delta_all[:, sl3], vrow(R_ONW, DG, g * DG), vrow(R_ONB, DG, g * DG),
               delta_all[:, sl3], DG)
        nc.vector.tensor_mul(out=delta_all[:], in0=delta_all[:], in1=sz_all[:])

        # out_proj per group
        yzT = transpose_in(delta_all[:], C, tag="yzT8")
        for g in range(4):
            ys_p = pm.tile([B, DG], f32, tag="pm")
            for t in range(2):
                nc.tensor.matmul(ys_p[:], lhsT=yzT[:, 2 * g + t, :],
                                 rhs=opw[:, 2 * g + t, :],
                                 start=(t == 0), stop=(t == 1))
            nc.vector.tensor_copy(out=ycat[:, g * DG:(g + 1) * DG], in_=ys_p[:])

        tap(2, ycat[:])
        # y2 = ycat * skip * xn * se ; skip_scale is baked into xn's use here
        # via a host-premultiplied R_CW? no: skip_scale folded into vrow(R_D)
        # would be wrong; apply as scalar mul (host passes it in sel row? no)
        # -- host bakes skip_scale into gm_proj_w? also wrong (LN in between).
        # Apply with tensor_scalar: skip is a compile-time-unknown scalar, so
        # it rides vecs[R_MISC,0] as a per-partition [B,1] scalar AP.
        nc.vector.tensor_scalar_mul(out=ycat[:], in0=ycat[:],
                                    scalar1=vecs[:, R_MISC * 1024:R_MISC * 1024 + 1])
        nc.vector.tensor_mul(out=ycat[:], in0=ycat[:], in1=xn[:])
        nc.vector.tensor_mul(out=ycat[:], in0=ycat[:], in1=se_t[:])
        y3 = a1k.tile([B, C], f32, tag="a1k")
        ln(ycat[:], vrow(R_GMW), vrow(R_GMB), y3[:], C)
        tap(3, y3[:])

        # a = y3 @ gm_proj + b   (streamed bf16 chunks, both 512-col halves
        # accumulated in one [B,1024] PSUM so each chunk is consumed once)
        y3T = transpose_in(y3[:], C, tag="y3T")
        a_p = pm.tile([B, C], f32, tag="pm")
        for cnum in range(4):
            for t in range(2):
                kt = 2 * cnum + t
                for n in range(2):
                    nc.tensor.matmul(a_p[:, n * 512:(n + 1) * 512],
                                     lhsT=y3T[:, kt, :],
                                     rhs=gmw_c[cnum][:, t, n * 512:(n + 1) * 512],
                                     start=(kt == 0), stop=(kt == 7))
        a_s = a1k.tile([B, C], f32, tag="a1k")
        nc.vector.tensor_add(out=a_s[:], in0=a_p[:], in1=vrow(R_GMPB))
        aln = a1k.tile([B, C], f32, tag="a1k")
        ln(a_s[:], vrow(R_N1W), vrow(R_N1B), aln[:], C)
        cls1 = singles.tile([B, C], f32, tag="cls1")
        nc.vector.tensor_add(out=cls1[:], in0=cls_t[:], in1=aln[:])
        tap(4, cls1[:])

        # MLP (fc1 col-shard, fc2 row-shard, AllToAll + local reduce)
        h = a1k.tile([B, C], f32, tag="a1k")
        ln(cls1[:], vrow(R_N2W), vrow(R_N2B), h[:], C)
        hT = transpose_in(h[:], C, tag="hT")
        h1_p = pm.tile([B, FC1_SH], f32, tag="pm")
        for cnum in range(2):
            for t in range(4):
                kt = 4 * cnum + t
                nc.tensor.matmul(h1_p[:], lhsT=hT[:, kt, :],
                                 rhs=fc1_c[cnum][:, t, :],
                                 start=(kt == 0), stop=(kt == 7))
        h1 = a512.tile([B, FC1_SH], f32, tag="h1")
        nc.vector.tensor_add(out=h1[:], in0=h1_p[:], in1=vrow(R_MISC, FC1_SH, OFF_FC1B))
        # exact gelu: x * (0.5 + 0.5*erf(x/sqrt(2)))
        ger = a512.tile([B, FC1_SH], f32, tag="h1")
        nc.scalar.activation(out=ger[:], in_=h1[:], func=AF.Erf,
                             scale=float(1.0 / np.sqrt(2.0)))
        nc.scalar.activation(out=ger[:], in_=ger[:], func=AF.Copy,
                             bias=0.5, scale=0.5)
        nc.vector.tensor_mul(out=h1[:], in0=h1[:], in1=ger[:])
        tap(5, h1[:], FC1_SH)

        h1T = transpose_in(h1[:], FC1_SH, tag="h1T")
        p_p = pm.tile([B, C], f32, tag="pm")
        for cnum in range(2):
            for t in range(2):
                kt = 2 * cnum + t
                for n in range(2):
                    nc.tensor.matmul(p_p[:, n * 512:(n + 1) * 512],
                                     lhsT=h1T[:, kt, :],
                                     rhs=fc2_c[cnum][:, t, n * 512:(n + 1) * 512],
                                     start=(kt == 0), stop=(kt == 3))
        p_s = a1k.tile([B, C], f32, tag="a1k")
        nc.vector.tensor_copy(out=p_s[:], in_=p_p[:])

        # select this core's 2 batch rows of cls1 early (runs before the MLP
        # partials finish); fp32 one-hot matmul keeps the rows exact.
        or_p = pm.tile([BPC, C], f32, tag="pm")
        for n in range(2):
            nc.tensor.matmul(or_p[:, n * 512:(n + 1) * 512], lhsT=sel_t[:, 0:BPC],
                             rhs=cls1[:, n * 512:(n + 1) * 512], start=True, stop=True)
        orow = tiny.tile([BPC, C], f32, tag="orow")
        nc.vector.tensor_copy(out=orow[:], in_=or_p[:])

        # AllToAll: my input chunk j = my partials for core j's rows; my
        # output chunk p = core p's partials for my rows.  Local reduce via
        # a [16,2] summation matmul, fp32.
        USE_A2A = False
        cc_in = dram.tile([B, C], f32, tag="cc_in")
        nc.scalar.dma_start(out=cc_in[:], in_=p_s[:])
        # copy-C: scalar-ring copy share, emitted after the chain's scalar
        # ops so its issues never delay them; runs while the collective is
        # in flight and the sync ring drains its share.
        copy_stream(nc.scalar, cpc,
                    list(range(2 * SYNC_N + B_N, 2 * NALL)), LOOKB, half=True)
        if USE_A2A:
            cc_out = dram.tile([B, C], f32, tag="cc_out")
            nc.gpsimd.collective_compute(
                "AllToAll", mybir.AluOpType.bypass,
                replica_groups=[list(range(NCORES))],
                ins=[cc_in[:].opt()], outs=[cc_out[:].opt()],
            )
            ato = tiny.tile([B, C], f32, tag="ato")
            nc.scalar.dma_start(out=ato[:], in_=cc_out[:])
            if dbg_h is not None:
                nc.scalar.dma_start(out=dbg_h[6, :, :], in_=ato[:])
            h2_p = pm.tile([BPC, C], f32, tag="pm")
            for n in range(2):
                nc.tensor.matmul(h2_p[:, n * 512:(n + 1) * 512], lhsT=sel_t[:, BPC:],
                                 rhs=ato[:, n * 512:(n + 1) * 512], start=True, stop=True)
            h2_sl = h2_p[:]
        else:
            cc_out = dram.tile([BPC, C], f32, tag="cc_out")
            nc.gpsimd.collective_compute(
                "ReduceScatter", mybir.AluOpType.add,
                replica_groups=[list(range(NCORES))],
                ins=[cc_in[:].opt()], outs=[cc_out[:].opt()],
            )
            h2 = tiny.tile([BPC, C], f32, tag="ato")
            nc.scalar.dma_start(out=h2[:], in_=cc_out[:])
            if dbg_h is not None:
                nc.scalar.dma_start(out=dbg_h[6, :BPC, :], in_=h2[:])
            h2_sl = h2[:]

        # out rows = cls1_rows + mlp_rows + fc2_b
        nc.vector.tensor_add(out=orow[:], in0=orow[:], in1=h2_sl)
        nc.vector.tensor_add(out=orow[:], in0=orow[:], in1=vrow(R_FC2B)[:BPC, :])
        nc.scalar.dma_start(out=out_h[:, 0, :], in_=orow[:])

    nc.compile()
    return nc


def _prepare_in_maps(inputs):
    import ml_dtypes

    x = _f32(inputs["x"])
    cls_all = _f32(x[:, 0, :])
    cw_center = _f32(inputs["ss_conv_w"])[:, :, 1, 1]  # [4, 256]
    skip = float(_f32(inputs["skip_scale"]).reshape(-1)[0])

    base_vecs = np.zeros((NV, 1024), np.float32)
    base_vecs[R_GMW] = _f32(inputs["gm_norm_w"])
    base_vecs[R_GMB] = _f32(inputs["gm_norm_b"])
    base_vecs[R_SE2B] = _f32(inputs["se_fc2_b"])
    base_vecs[R_N1W] = _f32(inputs["norm1_w"])
    base_vecs[R_N1B] = _f32(inputs["norm1_b"])
    base_vecs[R_N2W] = _f32(inputs["norm2_w"])
    base_vecs[R_N2B] = _f32(inputs["norm2_b"])
    base_vecs[R_FC2B] = _f32(inputs["mlp_fc2_b"])
    base_vecs[R_GMPB] = _f32(inputs["gm_proj_b"])
    base_vecs[R_CW] = cw_center.reshape(-1)
    base_vecs[R_CB] = _f32(inputs["ss_conv_b"]).reshape(-1)
    base_vecs[R_DTB] = _f32(inputs["ss_dt_b"]).reshape(-1)
    base_vecs[R_D] = _f32(inputs["ss_D"]).reshape(-1)
    base_vecs[R_ONW] = _f32(inputs["ss_out_norm_w"]).reshape(-1)
    base_vecs[R_ONB] = _f32(inputs["ss_out_norm_b"]).reshape(-1)
    base_vecs[R_MISC, OFF_SE1B:OFF_SE1B + RED] = _f32(inputs["se_fc1_b"])
    base_vecs[R_MISC, 0] = skip

    fc1_w = _f32(inputs["mlp_fc1_w"])
    fc1_b = _f32(inputs["mlp_fc1_b"])
    fc2_w = _f32(inputs["mlp_fc2_w"])

    def _w(a):
        return np.ascontiguousarray(_f32(a).astype(ml_dtypes.bfloat16))

    def _pmaj(a, n):
        # [T*128, n] -> [128, T, n]: partition-contiguous device layout
        a = np.asarray(a).reshape(-1, 128, n)
        return np.ascontiguousarray(a.transpose(1, 0, 2))

    # selsum[2p+r, r] = 1: local reduction of the AllToAll result
    selsum = np.zeros((B, BPC), np.float32)
    for p in range(NCORES):
        for r in range(BPC):
            selsum[p * BPC + r, r] = 1.0

    shared = {
        "cls_all": cls_all,
        "ident16": np.eye(B, dtype=np.float32),
        "se1w": _pmaj(_w(inputs["se_fc1_w"]), RED),
        "se2w": _w(inputs["se_fc2_w"]),
        "ipw": _pmaj(_w(inputs["ss_in_proj"]).reshape(4 * DG, 2 * DG), 2 * DG),
        "xpw": _pmaj(_w(inputs["ss_x_proj"]).reshape(4 * DG, DTRANK + 2), DTRANK + 2),
        "dtw": np.ascontiguousarray(_w(inputs["ss_dt_w"]).transpose(1, 0, 2)),
        "opw": _pmaj(_w(inputs["ss_out_proj"]).reshape(4 * DG, DG), DG),
        "gmw": _pmaj(_w(inputs["gm_proj_w"]), C),
    }

    in_maps = []
    for i in range(NCORES):
        vecs = base_vecs.copy()
        vecs[R_MISC, OFF_FC1B:OFF_FC1B + FC1_SH] = fc1_b[i * FC1_SH:(i + 1) * FC1_SH]
        sel = np.zeros((B, 2 * BPC), np.float32)
        for j in range(BPC):
            sel[i * BPC + j, j] = 1.0
        sel[:, BPC:] = selsum
        m = dict(shared)
        m.update({
            "xs": np.ascontiguousarray(x[i * BPC:(i + 1) * BPC]),
            "sel": sel,
            "vecs": np.ascontiguousarray(
                np.broadcast_to(vecs.reshape(1, -1), (B, NV * 1024))),
            "fc1s": _pmaj(_w(fc1_w[:, i * FC1_SH:(i + 1) * FC1_SH]), FC1_SH),
            "fc2s": _pmaj(_w(fc2_w[i * FC2_SH:(i + 1) * FC2_SH, :]), C),
        })
        in_maps.append(m)
    return in_maps


def _install_trace_shims():
    """This image lacks ``antenv.axon_hooks`` and fish-bucket access; stub in
    the ctypes NTFF hook from trn_boot and make artifact upload a no-op."""
    import sys
    import types

    import concourse.bass_utils as bu

    bu.upload_artifacts = lambda tmpdir: f"local:{tmpdir}"
    if "antenv.axon_hooks" not in sys.modules:
        from trn_agent_boot.trn_boot import _ntff_profile_via_ctypes

        mod = types.ModuleType("antenv.axon_hooks")
        hook = _ntff_profile_via_ctypes("/opt/axon/libaxon_pjrt.so")
        mod.get_axon_ntff_profile_hook = lambda: hook
        mod.set_axon_ntff_profile_hook = lambda h: None
        sys.modules["antenv.axon_hooks"] = mod
        import antenv

        antenv.axon_hooks = mod


def kernel(**inputs):
    global LAST_RESULT
    from concourse.bass_utils import run_bass_kernel_spmd

    key = "dbg" if DEBUG_TAPS else "plain"
    if key not in _CACHE:
        _CACHE[key] = _build(DEBUG_TAPS)
    nc = _CACHE[key]

    kwargs = {}
    if TRACE:
        _install_trace_shims()
        tdir = "/root/problem/.trace_" + key
        import os
        import shutil

        shutil.rmtree(tdir, ignore_errors=True)
        os.makedirs(tdir, exist_ok=True)
        kwargs = {"tmpdir": tdir}

    in_maps = _prepare_in_maps(inputs)
    res = run_bass_kernel_spmd(nc, in_maps, list(range(NCORES)), trace=TRACE, **kwargs)
    LAST_RESULT = res
    out = np.concatenate([res.results[i]["out"] for i in range(NCORES)], axis=0)
    return out


# revision 21
# speedup vs baseline: 1.0503x; 1.0503x over previous
"""Trainium2 Bass kernel for nn_ClassBlock (dense_transformer, memory regime).

Strategy
--------
The ClassBlock only transforms x[:, 0, :] (the cls token); x[:, 1:, :] passes
through untouched.  The kernel is therefore dominated by a 256 MB HBM->HBM
copy.  Sharding:
  * batch-parallel over 8 cores (2 batches/core) for the pass-through copy,
    split across two DMA streams (SP HWDGE ring + SWDGE) to saturate HBM,
  * the cls compute ([16,1024] activations) is replicated on every core,
    except the heavy MLP weights: fc1 is column-sharded, fc2 row-sharded,
  * MLP partials are exchanged with an AllToAll (pure p2p, much lower latency
    than ReduceScatter) and reduced locally on the PE,
  * each core writes only its own 2 batch rows of the cls result.
All matmuls run in bf16 (4x PE rate, half the weight HBM bytes); the row
select of cls1 stays fp32 (exact).  LayerNorm runs entirely on the vector
engine (bit-trick rsqrt + Newton) so the ACT LUT never thrashes.
L=1 structural simplifications (3x3 'SAME' depthwise conv on a 1x1 map ==
center tap; selective scan with L=1, h0=0 == dBu*Cs + D*u).
"""

import numpy as np

B, NTOK, C = 16, 4097, 1024
NCORES = 8
BPC = B // NCORES            # batches per core
DG = C // 4                  # 256 per-group channels
DTRANK = 16
HID = 4 * C                  # 4096
RED = C // 16                # 64
FC1_SH = HID // NCORES       # 512 fc1 column shard
FC2_SH = HID // NCORES       # 512 fc2 row shard
EPS = 1e-5

# packed per-channel vector blob rows (each row = 1024 f32)
R_GMW, R_GMB, R_SE2B, R_N1W, R_N1B, R_N2W, R_N2B, R_FC2B, R_GMPB = range(9)
R_CW, R_CB, R_DTB, R_D, R_ONW, R_ONB, R_MISC = range(9, 16)
NV = 16
# misc row layout: [64:128]=se_fc1_b, [512:1024]=fc1_b shard
OFF_SE1B = 64
OFF_FC1B = 512

# copy split: SYNC_N one-MB tiles on the SP HWDGE ring; B_N half-MB tiles on
# SWDGE (issued early, sized small so the collective dispatch behind them
# fires with little inter-core skew); C_N half-MB tiles on the ACT HWDGE
# ring after the cls chain's scalar ops retire.
CPF = 2048                   # 8 KB/partition per sync staging tile
CPH = 1024                   # 4 KB/partition per B/C staging tile
NCP = (NTOK - 1) * C // (128 * CPF)  # 16 tiles per batch row
NALL = NCP * BPC             # 32 one-MB tile slots
SYNC_N = 22
B_N = 8                      # half-MB tiles (4 MB) for SWDGE
C_N = 12                     # half-MB tiles (6 MB) for scalar post-chain
LOOKA = 4                    # sync-stream load lookahead
LOOKB = 1                    # swdge-stream load lookahead

DEBUG_TAPS = False

_CACHE = {}
LAST_RESULT = None
TRACE = False


def _f32(a):
    return np.ascontiguousarray(np.asarray(a, dtype=np.float32))


def _build(debug_taps):
    import concourse.bass as bass
    import concourse.tile as tile
    from concourse import bacc, mybir

    f32 = mybir.dt.float32
    i32 = mybir.dt.int32
    wdt = mybir.dt.bfloat16
    AF = mybir.ActivationFunctionType
    ALU = mybir.AluOpType

    # Bacc (not plain Bass): its compile() legalizes to <=1 sync wait per
    # instruction (generate_event_semaphores), which TRN2 codegen requires.
    nc = bacc.Bacc("TRN2", target_bir_lowering=False, num_devices=NCORES)

    # ---- I/O ------------------------------------------------------------
    xs_h = nc.dram_tensor("xs", [BPC, NTOK, C], f32, kind="ExternalInput")
    cls_h = nc.dram_tensor("cls_all", [B, C], f32, kind="ExternalInput")
    sel_h = nc.dram_tensor("sel", [B, 2 * BPC], f32, kind="ExternalInput")
    id_h = nc.dram_tensor("ident16", [B, B], f32, kind="ExternalInput")
    vecs_h = nc.dram_tensor("vecs", [B, NV * 1024], f32, kind="ExternalInput")
    # weights arrive pre-relaid by the host as [128, tiles, N] (partition
    # contiguous) — partition-scattered rearranges at load time make HWDGE
    # emit thousands of tiny descriptors (~28 GB/s measured)
    se1w_h = nc.dram_tensor("se1w", [128, 8, RED], wdt, kind="ExternalInput")
    se2w_h = nc.dram_tensor("se2w", [RED, C], wdt, kind="ExternalInput")
    ipw_h = nc.dram_tensor("ipw", [128, 8, 512], wdt, kind="ExternalInput")
    xpw_h = nc.dram_tensor("xpw", [128, 8, DTRANK + 2], wdt, kind="ExternalInput")
    dtw_h = nc.dram_tensor("dtw", [DTRANK, 4, DG], wdt, kind="ExternalInput")
    opw_h = nc.dram_tensor("opw", [128, 8, DG], wdt, kind="ExternalInput")
    gmw_h = nc.dram_tensor("gmw", [128, 8, C], wdt, kind="ExternalInput")
    fc1_h = nc.dram_tensor("fc1s", [128, 8, FC1_SH], wdt, kind="ExternalInput")
    fc2_h = nc.dram_tensor("fc2s", [128, 4, C], wdt, kind="ExternalInput")
    out_h = nc.dram_tensor("out", [BPC, NTOK, C], f32, kind="ExternalOutput")
    dbg_h = None
    if debug_taps:
        dbg_h = nc.dram_tensor("dbg", [8, B, C], f32, kind="ExternalOutput")

    from contextlib import ExitStack

    with tile.TileContext(nc) as tc, ExitStack() as ctx:
        singles = ctx.enter_context(tc.tile_pool(name="singles", bufs=1))
        wst = ctx.enter_context(tc.tile_pool(name="wst", bufs=8))
        a1k = ctx.enter_context(tc.tile_pool(name="a1k", bufs=2))
        a512 = ctx.enter_context(tc.tile_pool(name="a512", bufs=2))
        tiny = ctx.enter_context(tc.tile_pool(name="tiny", bufs=1))
        tp = ctx.enter_context(tc.tile_pool(name="tp", bufs=1))
        stats = ctx.enter_context(tc.tile_pool(name="stats", bufs=3))
        ppt = ctx.enter_context(tc.tile_pool(name="ppt", bufs=4, space="PSUM"))
        pm = ctx.enter_context(tc.tile_pool(name="pm", bufs=2, space="PSUM"))
        dram = ctx.enter_context(tc.tile_pool(name="dram", bufs=1, space="DRAM"))
        cpa = ctx.enter_context(tc.tile_pool(name="cpa", bufs=4))
        cpb = ctx.enter_context(tc.tile_pool(name="cpb", bufs=2))
        cpc = cpb  # B (gpsimd, 0-100us) and C (scalar, 150us+) never overlap

        # ---- the big pass-through copy (bulk of the kernel) -------------
        # DRAM->DRAM DMA is latency-bound, so stage through SBUF.  Tiles
        # 0..SYNC_N-1 ride the SP HWDGE ring; the rest ride SWDGE (issued
        # after the big-weight loads below, sized to finish issuing before
        # the collective dispatch at the end of the gpsimd stream).
        xs_flat = xs_h[:].rearrange("b t c -> b (t c)")
        out_flat = out_h[:].rearrange("b t c -> b (t c)")
        srcs, dsts, srcs_h, dsts_h = [], [], [], []
        for b in range(BPC):
            srcs.append(xs_flat[b, C:].rearrange("(n p f) -> n p f", p=128, f=CPF))
            dsts.append(out_flat[b, C:].rearrange("(n p f) -> n p f", p=128, f=CPF))
            srcs_h.append(xs_flat[b, C:].rearrange("(n p f) -> n p f", p=128, f=CPH))
            dsts_h.append(out_flat[b, C:].rearrange("(n p f) -> n p f", p=128, f=CPH))

        def copy_stream(eng, pool, idxs, looka, half=False):
            ss, dd, npr, w = ((srcs_h, dsts_h, 2 * NCP, CPH) if half
                             else (srcs, dsts, NCP, CPF))
            tiles = {}
            n_ = len(idxs)
            for k in range(n_ + looka):
                if k < n_:
                    t = pool.tile([128, w], f32, tag="cp")
                    i = idxs[k]
                    eng.dma_start(out=t[:], in_=ss[i // npr][i % npr])
                    tiles[k] = t
                if k >= looka:
                    i = idxs[k - looka]
                    eng.dma_start(out=dd[i // npr][i % npr],
                                  in_=tiles.pop(k - looka)[:])


        # ---- small weights / activations: scalar (ACT HWDGE) queue ------
        # Issued first so the cls chain can start within a few us.
        vecs = singles.tile([B, NV * 1024], f32, tag="vecs")
        nc.scalar.dma_start(out=vecs[:], in_=vecs_h[:])
        cls_t = singles.tile([B, C], f32, tag="cls")
        nc.scalar.dma_start(out=cls_t[:], in_=cls_h[:])
        sel_t = singles.tile([B, 2 * BPC], f32, tag="sel")
        nc.scalar.dma_start(out=sel_t[:], in_=sel_h[:])
        ident = singles.tile([B, B], f32, tag="ident")
        nc.scalar.dma_start(out=ident[:], in_=id_h[:])
        se1w = singles.tile([128, 8, RED], wdt, tag="se1w")
        nc.scalar.dma_start(out=se1w[:], in_=se1w_h[:])
        se2w = singles.tile([RED, 2, 512], wdt, tag="se2w")
        nc.scalar.dma_start(out=se2w[:], in_=se2w_h[:].rearrange("k (c n) -> k c n", c=2))
        xpw = singles.tile([128, 8, DTRANK + 2], wdt, tag="xpw")
        nc.scalar.dma_start(out=xpw[:], in_=xpw_h[:])
        dtw = singles.tile([DTRANK, 4, DG], wdt, tag="dtw")
        nc.scalar.dma_start(out=dtw[:], in_=dtw_h[:])
        ipw = singles.tile([128, 8, 512], wdt, tag="ipw")
        nc.scalar.dma_start(out=ipw[:], in_=ipw_h[:])

        # ---- big weights: SWDGE, then copy-B, then the collective -------
        opw = singles.tile([128, 8, DG], wdt, tag="opw")
        nc.gpsimd.dma_start(out=opw[:], in_=opw_h[:])
        gmw_r = gmw_h[:]     # [128, 8, 1024]
        fc1_r = fc1_h[:]     # [128, 8, 512]
        fc2_r = fc2_h[:]     # [128, 4, 1024]
        gmw_c = []
        for cnum in range(4):
            t = wst.tile([128, 2, 1024], wdt, tag="wst")
            nc.gpsimd.dma_start(out=t[:], in_=gmw_r[:, 2 * cnum:2 * cnum + 2, :])
            gmw_c.append(t)
        fc1_c = []
        for cnum in range(2):
            t = wst.tile([128, 4, 512], wdt, tag="wst")
            nc.gpsimd.dma_start(out=t[:], in_=fc1_r[:, 4 * cnum:4 * cnum + 4, :])
            fc1_c.append(t)
        fc2_c = []
        for cnum in range(2):
            t = wst.tile([128, 2, 1024], wdt, tag="wst")
            nc.gpsimd.dma_start(out=t[:], in_=fc2_r[:, 2 * cnum:2 * cnum + 2, :])
            fc2_c.append(t)

        # sync-ring copy emitted AFTER every weight load: the 8 DMAHW lane
        # slots are handed out in emission order, and a weight load stuck
        # behind a 1 MB copy tile on its lane trickles in ~5us steps.
        copy_stream(nc.sync, cpa, list(range(SYNC_N)), LOOKA)

        # copy-B: all weight chunks above issue instantly (wst bufs=8, no
        # recycling), so nothing here is gated on chain progress; keeping
        # copy-B small bounds the inter-core skew of the collective dispatch
        # that follows it in this stream.
        copy_stream(nc.gpsimd, cpb,
                    list(range(2 * SYNC_N, 2 * SYNC_N + B_N)), LOOKB, half=True)

        def vrow(row, n=1024, off=0):
            return vecs[:, row * 1024 + off: row * 1024 + off + n]

        # ---- helpers -----------------------------------------------------
        QK = 0x5F3759DF

        def ln(x_sl, w_sl, b_sl, out_sl, cdim):
            # fully on the vector engine: bn stats + bit-trick rsqrt with 3
            # Newton steps + fused (x-mean)*rstd; no ACT LUT traffic.
            nsub = max(1, cdim // 512)
            if nsub == 1:
                st = stats.tile([B, 6], f32, tag="st6")
                nc.vector.bn_stats(out=st[:], in_=x_sl)
            else:
                st = stats.tile([B, nsub, 6], f32, tag="st26")
                for s in range(nsub):
                    nc.vector.bn_stats(out=st[:, s, :], in_=x_sl[:, s * 512:(s + 1) * 512])
            mv = stats.tile([B, 2], f32, tag="mv")
            nc.vector.bn_aggr(out=mv[:], in_=st[:])
            q = stats.tile([B, 2], f32, tag="qrs")     # col0 = var+eps, col1 = y
            nc.vector.tensor_scalar_add(out=q[:, 0:1], in0=mv[:, 1:2], scalar1=EPS)
            qi = q[:].bitcast(i32)
            nc.vector.tensor_scalar(out=qi[:, 1:2], in0=qi[:, 0:1], scalar1=1,
                                    scalar2=None, op0=ALU.arith_shift_right)
            nc.vector.tensor_scalar(out=qi[:, 1:2], in0=qi[:, 1:2], scalar1=QK,
                                    scalar2=-1, op0=ALU.subtract, op1=ALU.mult)
            a = stats.tile([B, 1], f32, tag="nwt")
            for _ in range(2):
                nc.vector.tensor_mul(out=a[:], in0=q[:, 1:2], in1=q[:, 1:2])
                nc.vector.tensor_mul(out=a[:], in0=a[:], in1=q[:, 0:1])
                nc.vector.tensor_scalar(out=a[:], in0=a[:], scalar1=-0.5,
                                        scalar2=1.5, op0=ALU.mult, op1=ALU.add)
                nc.vector.tensor_mul(out=q[:, 1:2], in0=q[:, 1:2], in1=a[:])
            nc.vector.tensor_scalar(out=out_sl, in0=x_sl, scalar1=mv[:, 0:1],
                                    scalar2=q[:, 1:2], op0=ALU.subtract, op1=ALU.mult)
            nc.vector.tensor_mul(out=out_sl, in0=out_sl, in1=w_sl)
            nc.vector.tensor_add(out=out_sl, in0=out_sl, in1=b_sl)

        def transpose_in(x_sl, cdim, tag="tp"):
            # [16, cdim] (sbuf) -> [128, cdim//128, 16] (sbuf, bf16)
            kt = cdim // 128
            xT = tp.tile([128, kt, B], wdt, tag=tag)
            for t in range(kt):
                pt = ppt.tile([128, B], f32, tag="pt")
                nc.tensor.transpose(pt[:], x_sl[:, t * 128:(t + 1) * 128], ident[:])
                nc.vector.tensor_copy(out=xT[:, t, :], in_=pt[:])
            return xT

        def tap(i, src_sl, n=C):
            if dbg_h is not None:
                nc.scalar.dma_start(out=dbg_h[i, :, :n], in_=src_sl)

        # ---- cls chain ---------------------------------------------------
        xn = singles.tile([B, C], f32, tag="xn")
        ln(cls_t[:], vrow(R_GMW), vrow(R_GMB), xn[:], C)
        tap(0, xn[:])
        xnT = transpose_in(xn[:], C, tag="xnT_p")

        # SE block
        seh_p = pm.tile([B, RED], f32, tag="pm")
        for t in range(8):
            nc.tensor.matmul(seh_p[:], lhsT=xnT[:, t, :], rhs=se1w[:, t, :],
                             start=(t == 0), stop=(t == 7))
        seh = tiny.tile([B, RED], f32, tag="seh")
        nc.vector.tensor_add(out=seh[:], in0=seh_p[:], in1=vrow(R_MISC, RED, OFF_SE1B))
        nc.scalar.activation(out=seh[:], in_=seh[:], func=AF.Relu)
        pt = ppt.tile([128, B], f32, tag="pt")
        nc.tensor.transpose(pt[:RED, :], seh[:], ident[:])
        sehT = tiny.tile([RED, B], wdt, tag="sehT")
        nc.vector.tensor_copy(out=sehT[:], in_=pt[:RED, :])
        se_p = pm.tile([B, C], f32, tag="pm")
        for n in range(2):
            nc.tensor.matmul(se_p[:, n * 512:(n + 1) * 512], lhsT=sehT[:],
                             rhs=se2w[:, n, :], start=True, stop=True)
        se_t = singles.tile([B, C], f32, tag="se")
        nc.vector.tensor_add(out=se_t[:], in0=se_p[:], in1=vrow(R_SE2B))
        nc.scalar.activation(out=se_t[:], in_=se_t[:], func=AF.Sigmoid)
        tap(1, se_t[:])

        # SS2D groups — sigmoid phase: in_proj, u = silu(xs*cw+cb), sz = silu(z)
        ycat = singles.tile([B, C], f32, tag="ycat")
        u_all = singles.tile([B, C], f32, tag="uall")
        sz_all = singles.tile([B, C], f32, tag="szall")
        for g in range(4):
            xz_p = pm.tile([B, 2 * DG], f32, tag="pm")
            for t in range(2):
                gt = 2 * g + t
                nc.tensor.matmul(xz_p[:], lhsT=xnT[:, gt, :], rhs=ipw[:, gt, :],
                                 start=(t == 0), stop=(t == 1))
            sl = slice(g * DG, (g + 1) * DG)
            nc.vector.tensor_copy(out=u_all[:, sl], in_=xz_p[:, :DG])
            nc.vector.tensor_copy(out=sz_all[:, sl], in_=xz_p[:, DG:])
        nc.vector.tensor_mul(out=u_all[:], in0=u_all[:], in1=vrow(R_CW))
        nc.vector.tensor_add(out=u_all[:], in0=u_all[:], in1=vrow(R_CB))
        sgt = a1k.tile([B, C], f32, tag="a1k")
        nc.scalar.activation(out=sgt[:], in_=u_all[:], func=AF.Sigmoid)
        nc.vector.tensor_mul(out=u_all[:], in0=u_all[:], in1=sgt[:])
        sgt2 = a1k.tile([B, C], f32, tag="a1k")
        nc.scalar.activation(out=sgt2[:], in_=sz_all[:], func=AF.Sigmoid)
        nc.vector.tensor_mul(out=sz_all[:], in0=sz_all[:], in1=sgt2[:])

        # x_dbl, delta = softplus, y, out-norm LN
        uT = transpose_in(u_all[:], C, tag="uT8")
        delta_all = singles.tile([B, C], f32, tag="dall")
        bcs = []
        for g in range(4):
            xdb_p = pm.tile([B, DTRANK + 2], f32, tag="pm")
            for t in range(2):
                nc.tensor.matmul(xdb_p[:], lhsT=uT[:, 2 * g + t, :],
                                 rhs=xpw[:, 2 * g + t, :],
                                 start=(t == 0), stop=(t == 1))
            xdb = tiny.tile([B, DTRANK + 2], f32, tag="xdb")
            nc.vector.tensor_copy(out=xdb[:], in_=xdb_p[:])
            bc = stats.tile([B, 1], f32, tag="bc")
            nc.vector.tensor_mul(out=bc[:], in0=xdb[:, DTRANK:DTRANK + 1],
                                 in1=xdb[:, DTRANK + 1:DTRANK + 2])
            bcs.append(bc)
            pt2 = ppt.tile([128, B], f32, tag="pt")
            nc.tensor.transpose(pt2[:DTRANK, :], xdb[:, :DTRANK], ident[:])
            dtsT = tiny.tile([DTRANK, B], wdt, tag="dtsT")
            nc.vector.tensor_copy(out=dtsT[:], in_=pt2[:DTRANK, :])
            dl_p = pm.tile([B, DG], f32, tag="pm")
            nc.tensor.matmul(dl_p[:], lhsT=dtsT[:], rhs=dtw[:, g, :], start=True, stop=True)
            nc.vector.tensor_copy(out=delta_all[:, g * DG:(g + 1) * DG], in_=dl_p[:])
        nc.vector.tensor_add(out=delta_all[:], in0=delta_all[:], in1=vrow(R_DTB))
        # softplus(x) = relu(x) + ln(1 + exp(-|x|)); native Softplus LUT is
        # broken in this neuronx-cc build
        spt = a1k.tile([B, C], f32, tag="a1k")
        nc.scalar.activation(out=spt[:], in_=delta_all[:], func=AF.Abs)
        nc.scalar.activation(out=spt[:], in_=spt[:], func=AF.Exp, scale=-1.0)
        nc.vector.tensor_scalar_add(out=spt[:], in0=spt[:], scalar1=1.0)
        nc.scalar.activation(out=spt[:], in_=spt[:], func=AF.Ln)
        nc.scalar.activation(out=delta_all[:], in_=delta_all[:], func=AF.Relu)
        nc.vector.tensor_add(out=delta_all[:], in0=delta_all[:], in1=spt[:])
        # y = delta*u*(Bs*Cs) + D*u
        nc.vector.tensor_mul(out=delta_all[:], in0=delta_all[:], in1=u_all[:])
        for g in range(4):
            sl2 = slice(g * DG, (g + 1) * DG)
            nc.vector.tensor_scalar_mul(out=delta_all[:, sl2], in0=delta_all[:, sl2],
                                        scalar1=bcs[g][:])
        t2 = a1k.tile([B, C], f32, tag="a1k")
        nc.vector.tensor_mul(out=t2[:], in0=u_all[:], in1=vrow(R_D))
        nc.vector.tensor_add(out=delta_all[:], in0=delta_all[:], in1=t2[:])
        # per-group out-norm LN (stats over 256 channels), then * silu(z)
        for g in range(4):
            sl3 = slice(g * DG, (g + 1) * DG)
            ln(delta_all[:, sl3], vrow(R_ONW, DG, g * DG), vrow(R_ONB, DG, g * DG),
               delta_all[:, sl3], DG)
        nc.vector.tensor_mul(out=delta_all[:], in0=delta_all[:], in1=sz_all[:])

        # out_proj per group
        yzT = transpose_in(delta_all[:], C, tag="yzT8")
        for g in range(4):
            ys_p = pm.tile([B, DG], f32, tag="pm")
            for t in range(2):
                nc.tensor.matmul(ys_p[:], lhsT=yzT[:, 2 * g + t, :],
                                 rhs=opw[:, 2 * g + t, :],
                                 start=(t == 0), stop=(t == 1))
            nc.vector.tensor_copy(out=ycat[:, g * DG:(g + 1) * DG], in_=ys_p[:])

        tap(2, ycat[:])
        # y2 = ycat * skip * xn * se ; skip_scale is baked into xn's use here
        # via a host-premultiplied R_CW? no: skip_scale folded into vrow(R_D)
        # would be wrong; apply as scalar mul (host passes it in sel row? no)
        # -- host bakes skip_scale into gm_proj_w? also wrong (LN in between).
        # Apply with tensor_scalar: skip is a compile-time-unknown scalar, so
        # it rides vecs[R_MISC,0] as a per-partition [B,1] scalar AP.
        nc.vector.tensor_scalar_mul(out=ycat[:], in0=ycat[:],
                                    scalar1=vecs[:, R_MISC * 1024:R_MISC * 1024 + 1])
        nc.vector.tensor_mul(out=ycat[:], in0=ycat[:], in1=xn[:])
        nc.vector.tensor_mul(out=ycat[:], in0=ycat[:], in1=se_t[:])
        y3 = a1k.tile([B, C], f32, tag="a1k")
        ln(ycat[:], vrow(R_GMW), vrow(R_GMB), y3[:], C)
        tap(3, y3[:])

        # a = y3 @ gm_proj + b   (streamed bf16 chunks, both 512-col halves
        # accumulated in one [B,1024] PSUM so each chunk is consumed once)
        y3T = transpose_in(y3[:], C, tag="y3T")
        a_p = pm.tile([B, C], f32, tag="pm")
        for cnum in range(4):
            for t in range(2):
                kt = 2 * cnum + t
                for n in range(2):
                    nc.tensor.matmul(a_p[:, n * 512:(n + 1) * 512],
                                     lhsT=y3T[:, kt, :],
                                     rhs=gmw_c[cnum][:, t, n * 512:(n + 1) * 512],
                                     start=(kt == 0), stop=(kt == 7))
        a_s = a1k.tile([B, C], f32, tag="a1k")
        nc.vector.tensor_add(out=a_s[:], in0=a_p[:], in1=vrow(R_GMPB))
        aln = a1k.tile([B, C], f32, tag="a1k")
        ln(a_s[:], vrow(R_N1W), vrow(R_N1B), aln[:], C)
        cls1 = singles.tile([B, C], f32, tag="cls1")
        nc.vector.tensor_add(out=cls1[:], in0=cls_t[:], in1=aln[:])
        tap(4, cls1[:])

        # MLP (fc1 col-shard, fc2 row-shard, AllToAll + local reduce)
        h = a1k.tile([B, C], f32, tag="a1k")
        ln(cls1[:], vrow(R_N2W), vrow(R_N2B), h[:], C)
        hT = transpose_in(h[:], C, tag="hT")
        h1_p = pm.tile([B, FC1_SH], f32, tag="pm")
        for cnum in range(2):
            for t in range(4):
                kt = 4 * cnum + t
                nc.tensor.matmul(h1_p[:], lhsT=hT[:, kt, :],
                                 rhs=fc1_c[cnum][:, t, :],
                                 start=(kt == 0), stop=(kt == 7))
        h1 = a512.tile([B, FC1_SH], f32, tag="h1")
        nc.vector.tensor_add(out=h1[:], in0=h1_p[:], in1=vrow(R_MISC, FC1_SH, OFF_FC1B))
        # exact gelu: x * (0.5 + 0.5*erf(x/sqrt(2)))
        ger = a512.tile([B, FC1_SH], f32, tag="h1")
        nc.scalar.activation(out=ger[:], in_=h1[:], func=AF.Erf,
                             scale=float(1.0 / np.sqrt(2.0)))
        nc.scalar.activation(out=ger[:], in_=ger[:], func=AF.Copy,
                             bias=0.5, scale=0.5)
        nc.vector.tensor_mul(out=h1[:], in0=h1[:], in1=ger[:])
        tap(5, h1[:], FC1_SH)

        h1T = transpose_in(h1[:], FC1_SH, tag="h1T")
        p_p = pm.tile([B, C], f32, tag="pm")
        for cnum in range(2):
            for t in range(2):
                kt = 2 * cnum + t
                for n in range(2):
                    nc.tensor.matmul(p_p[:, n * 512:(n + 1) * 512],
                                     lhsT=h1T[:, kt, :],
                                     rhs=fc2_c[cnum][:, t, n * 512:(n + 1) * 512],
                                     start=(kt == 0), stop=(kt == 3))
        p_s = a1k.tile([B, C], f32, tag="a1k")
        nc.vector.tensor_copy(out=p_s[:], in_=p_p[:])

        # select this core's 2 batch rows of cls1 early (runs before the MLP
        # partials finish); fp32 one-hot matmul keeps the rows exact.
        or_p = pm.tile([BPC, C], f32, tag="pm")
        for n in range(2):
            nc.tensor.matmul(or_p[:, n * 512:(n + 1) * 512], lhsT=sel_t[:, 0:BPC],
                             rhs=cls1[:, n * 512:(n + 1) * 512], start=True, stop=True)
        orow = tiny.tile([BPC, C], f32, tag="orow")
        nc.vector.tensor_copy(out=orow[:], in_=or_p[:])

        # AllToAll: my input chunk j = my partials for core j's rows; my
        # output chunk p = core p's partials for my rows.  Local reduce via
        # a [16,2] summation matmul, fp32.
        USE_A2A = False
        cc_in = dram.tile([B, C], f32, tag="cc_in")
        nc.scalar.dma_start(out=cc_in[:], in_=p_s[:])
        # copy-C: scalar-ring copy share, emitted after the chain's scalar
        # ops so its issues never delay them; runs while the collective is
        # in flight and the sync ring drains its share.
        copy_stream(nc.scalar, cpc,
                    list(range(2 * SYNC_N + B_N, 2 * NALL)), LOOKB, half=True)
        if USE_A2A:
            cc_out = dram.tile([B, C], f32, tag="cc_out")
            nc.gpsimd.collective_compute(
                "AllToAll", mybir.AluOpType.bypass,
                replica_groups=[list(range(NCORES))],
                ins=[cc_in[:].opt()], outs=[cc_out[:].opt()],
            )
            ato = tiny.tile([B, C], f32, tag="ato")
            nc.scalar.dma_start(out=ato[:], in_=cc_out[:])
            if dbg_h is not None:
                nc.scalar.dma_start(out=dbg_h[6, :, :], in_=ato[:])
            h2_p = pm.tile([BPC, C], f32, tag="pm")
            for n in range(2):
                nc.tensor.matmul(h2_p[:, n * 512:(n + 1) * 512], lhsT=sel_t[:, BPC:],
                                 rhs=ato[:, n * 512:(n + 1) * 512], start=True, stop=True)
            h2_sl = h2_p[:]
        else:
            cc_out = dram.tile([BPC, C], f32, tag="cc_out")
            nc.gpsimd.collective_compute(
                "ReduceScatter", mybir.AluOpType.add,
                replica_groups=[list(range(NCORES))],
                ins=[cc_in[:].opt()], outs=[cc_out[:].opt()],
            )
            h2 = tiny.tile([BPC, C], f32, tag="ato")
            nc.scalar.dma_start(out=h2[:], in_=cc_out[:])
            if dbg_h is not None:
                nc.scalar.dma_start(out=dbg_h[6, :BPC, :], in_=h2[:])
            h2_sl = h2[:]

        # out rows = cls1_rows + mlp_rows + fc2_b
        nc.vector.tensor_add(out=orow[:], in0=orow[:], in1=h2_sl)
        nc.vector.tensor_add(out=orow[:], in0=orow[:], in1=vrow(R_FC2B)[:BPC, :])
        nc.scalar.dma_start(out=out_h[:, 0, :], in_=orow[:])

    nc.compile()
    return nc


def _prepare_in_maps(inputs):
    import ml_dtypes

    x = _f32(inputs["x"])
    cls_all = _f32(x[:, 0, :])
    cw_center = _f32(inputs["ss_conv_w"])[:, :, 1, 1]  # [4, 256]
    skip = float(_f32(inputs["skip_scale"]).reshape(-1)[0])

    base_vecs = np.zeros((NV, 1024), np.float32)
    base_vecs[R_GMW] = _f32(inputs["gm_norm_w"])
    base_vecs[R_GMB] = _f32(inputs["gm_norm_b"])
    base_vecs[R_SE2B] = _f32(inputs["se_fc2_b"])
    base_vecs[R_N1W] = _f32(inputs["norm1_w"])
    base_vecs[R_N1B] = _f32(inputs["norm1_b"])
    base_vecs[R_N2W] = _f32(inputs["norm2_w"])
    base_vecs[R_N2B] = _f32(inputs["norm2_b"])
    base_vecs[R_FC2B] = _f32(inputs["mlp_fc2_b"])
    base_vecs[R_GMPB] = _f32(inputs["gm_proj_b"])
    base_vecs[R_CW] = cw_center.reshape(-1)
    base_vecs[R_CB] = _f32(inputs["ss_conv_b"]).reshape(-1)
    base_vecs[R_DTB] = _f32(inputs["ss_dt_b"]).reshape(-1)
    base_vecs[R_D] = _f32(inputs["ss_D"]).reshape(-1)
    base_vecs[R_ONW] = _f32(inputs["ss_out_norm_w"]).reshape(-1)
    base_vecs[R_ONB] = _f32(inputs["ss_out_norm_b"]).reshape(-1)
    base_vecs[R_MISC, OFF_SE1B:OFF_SE1B + RED] = _f32(inputs["se_fc1_b"])
    base_vecs[R_MISC, 0] = skip

    fc1_w = _f32(inputs["mlp_fc1_w"])
    fc1_b = _f32(inputs["mlp_fc1_b"])
    fc2_w = _f32(inputs["mlp_fc2_w"])

    def _w(a):
        return np.ascontiguousarray(_f32(a).astype(ml_dtypes.bfloat16))

    def _pmaj(a, n):
        # [T*128, n] -> [128, T, n]: partition-contiguous device layout
        a = np.asarray(a).reshape(-1, 128, n)
        return np.ascontiguousarray(a.transpose(1, 0, 2))

    # selsum[2p+r, r] = 1: local reduction of the AllToAll result
    selsum = np.zeros((B, BPC), np.float32)
    for p in range(NCORES):
        for r in range(BPC):
            selsum[p * BPC + r, r] = 1.0

    shared = {
        "cls_all": cls_all,
        "ident16": np.eye(B, dtype=np.float32),
        "se1w": _pmaj(_w(inputs["se_fc1_w"]), RED),
        "se2w": _w(inputs["se_fc2_w"]),
        "ipw": _pmaj(_w(inputs["ss_in_proj"]).reshape(4 * DG, 2 * DG), 2 * DG),
        "xpw": _pmaj(_w(inputs["ss_x_proj"]).reshape(4 * DG, DTRANK + 2), DTRANK + 2),
        "dtw": np.ascontiguousarray(_w(inputs["ss_dt_w"]).transpose(1, 0, 2)),
        "opw": _pmaj(_w(inputs["ss_out_proj"]).reshape(4 * DG, DG), DG),
        "gmw": _pmaj(_w(inputs["gm_proj_w"]), C),
    }

    in_maps = []
    for i in range(NCORES):
        vecs = base_vecs.copy()
        vecs[R_MISC, OFF_FC1B:OFF_FC1B + FC1_SH] = fc1_b[i * FC1_SH:(i + 1) * FC1_SH]
        sel = np.zeros((B, 2 * BPC), np.float32)
        for j in range(BPC):
            sel[i * BPC + j, j] = 1.0
        sel[:, BPC:] = selsum
        m = dict(shared)
        m.update({
            "xs": np.ascontiguousarray(x[i * BPC:(i + 1) * BPC]),
            "sel": sel,
            "vecs": np.ascontiguousarray(
                np.broadcast_to(vecs.reshape(1, -1), (B, NV * 1024))),
            "fc1s": _pmaj(_w(fc1_w[:, i * FC1_SH:(i + 1) * FC1_SH]), FC1_SH),
            "fc2s": _pmaj(_w(fc2_w[i * FC2_SH:(i + 1) * FC2_SH, :]), C),
        })
        in_maps.append(m)
    return in_maps


def _install_trace_shims():
    """This image lacks ``antenv.axon_hooks`` and fish-bucket access; stub in
    the ctypes NTFF hook from trn_boot and make artifact upload a no-op."""
    import sys
    import types

    import concourse.bass_utils as bu

    bu.upload_artifacts = lambda tmpdir: f"local:{tmpdir}"
    if "antenv.axon_hooks" not in sys.modules:
        from trn_agent_boot.trn_boot import _ntff_profile_via_ctypes

        mod = types.ModuleType("antenv.axon_hooks")
        hook = _ntff_profile_via_ctypes("/opt/axon/libaxon_pjrt.so")
        mod.get_axon_ntff_profile_hook = lambda: hook
        mod.set_axon_ntff_profile_hook = lambda h: None
        sys.modules["antenv.axon_hooks"] = mod
        import antenv

        antenv.axon_hooks = mod


def kernel(**inputs):
    global LAST_RESULT
    from concourse.bass_utils import run_bass_kernel_spmd

    key = "dbg" if DEBUG_TAPS else "plain"
    if key not in _CACHE:
        _CACHE[key] = _build(DEBUG_TAPS)
    nc = _CACHE[key]

    kwargs = {}
    if TRACE:
        _install_trace_shims()
        tdir = "/root/problem/.trace_" + key
        import os
        import shutil

        shutil.rmtree(tdir, ignore_errors=True)
        os.makedirs(tdir, exist_ok=True)
        kwargs = {"tmpdir": tdir}

    in_maps = _prepare_in_maps(inputs)
    res = run_bass_kernel_spmd(nc, in_maps, list(range(NCORES)), trace=TRACE, **kwargs)
    LAST_RESULT = res
    out = np.concatenate([res.results[i]["out"] for i in range(NCORES)], axis=0)
    return out
